# revision 5
# baseline (speedup 1.0000x reference)
"""CrossCoderDecoder forward on 8 trn2 NeuronCores.

x[b,l,d] = sum_f f[b,f] * weight[l,f,d] + bias[l,d]
B=32, L=2, F=65536, D=768, fp32.

Sharding: the F (dict) axis is split 8 ways (8192 features per core).
Each core computes its partial [128, D] sums (partitions 0-63 = l0's
hi/lo batch pair, 64-127 = l1's); the host sums the 8 partials,
combines hi + lo/64, rescales, and adds the bias.

Precision: weights stream as float8e3 (E3M4), pre-scaled by 256 so the
Gaussian weight distribution sits in E3M4's normal range; end-to-end
error ~1.3e-2 relative (gate 2e-2), dominated by weight quantization.
f stays near-fp32 via an e3m4 hi/lo pair packed side by side in the
stationary operand ([128, 64] lhsT = 32 hi + 32 lo columns).

PE array usage (the v2 change): the baseline ran every matmul on
column-tile T0 of the (128,64)-tiled PE array -- out partitions 0-63,
half the array dark -- so the PE (46.8us busy) outlasted the DMA
stream (39.4us) by ~8us of pure tail.  Now each DMA chunk carries BOTH
l planes interleaved ([P, ko, L, D] image) and l1's matmuls target
PSUM partitions 64-127, i.e. column tile T1.  T0 and T1 stream
concurrently, halving PE wall time (~23us) so the PE stays hidden
under the DMA stream, and both l's finish together into one [128, 768]
PSUM image -> single cast -> two small stores.

DMA: ~18 chunks (two 256-row warmup chunks so the first matmul starts
~1us after the first chunk lands, 512-row = 786KB steady chunks, two
256-row tail chunks to shrink the last-chunk matmul tail), alternating
across both HWDGE rings (sync/scalar); f ships as f0 (ACT head, covers
the first 8 units) + f1 (SWDGE) exactly as in the baseline.  Measured
stream rate ~94% of the 358 GB/s per-core HBM cap.
"""

import numpy as np
import ml_dtypes

import concourse.bass as bass
import concourse.tile as tile
from concourse import bacc, mybir
from concourse import bass_utils

B, L, F, D = 32, 2, 65536, 768
NCORES = 8
FS = F // NCORES          # 8192 features per core
P = 128
# Chunk schedule in feature rows; each chunk carries both l planes
# (bytes per chunk = r * L * D).  Ramp-in for early matmul start,
# ramp-out so the last chunk's matmul tail is short.
CHUNK_SCHED = (256, 256) + (512,) * 14 + (256, 128, 128)
assert sum(CHUNK_SCHED) == FS
J = FS // P               # 64 k-subtiles ("units")
W_BUFS = 12               # weight tile double-buffering depth
NSPLITS = ((0, 512), (512, 768))  # PSUM-bank splits of D
WSCALE = 256.0            # weight pre-scale (exact power of 2)
FLSCALE = 64.0            # f-lo pre-scale (exact power of 2)
PREWARM = 10              # zero-matmuls warming the PE's HAM clock gate
F0_UNITS = 8              # f units in the early f transfer (covers c0-c2)

_F32 = mybir.dt.float32
_BF16 = mybir.dt.bfloat16
_FP8 = mybir.dt.float8e3
_FP8_NP = ml_dtypes.float8_e3m4

_cache = {}


def set_sched(sched: tuple, w_bufs: int | None = None, prewarm: int | None = None):
    """Adjust chunking (for tuning sweeps); drops the cached program."""
    global CHUNK_SCHED, W_BUFS, PREWARM
    CHUNK_SCHED = tuple(sched)
    assert sum(CHUNK_SCHED) == FS
    if w_bufs is not None:
        W_BUFS = w_bufs
    if prewarm is not None:
        PREWARM = prewarm
    _cache.clear()


def _build():
    """Build + schedule the (per-core identical) Bass program once."""
    nc = bacc.Bacc("TRN2", target_bir_lowering=False, debug=False)

    f0 = nc.dram_tensor("f0", [P, F0_UNITS, 2 * B], _FP8, kind="ExternalInput").ap()
    f1 = nc.dram_tensor(
        "f1", [P, J - F0_UNITS, 2 * B], _FP8, kind="ExternalInput"
    ).ap()
    # One dram tensor per chunk size class; w<r> holds every chunk of
    # r k-rows in schedule order, both l planes interleaved.
    sizes = sorted(set(CHUNK_SCHED))
    counts = {r: CHUNK_SCHED.count(r) for r in sizes}
    wt_dram = {
        r: nc.dram_tensor(
            f"w{r}", [counts[r], P, r // P, L, D], _FP8, kind="ExternalInput"
        ).ap()
        for r in sizes
    }
    # bf16 partial image: partitions 0-63 l0 (32 hi + 32 lo), 64-127 l1.
    out = nc.dram_tensor("out", [P, D], _BF16, kind="ExternalOutput").ap()

    with tile.TileContext(nc) as tc:
        with (
            tc.tile_pool(name="fpool", bufs=1) as fpool,
            tc.tile_pool(name="spool", bufs=1) as spool,
            tc.tile_pool(name="wpool", bufs=W_BUFS) as wpool,
            tc.tile_pool(name="opool", bufs=1) as opool,
            tc.tile_pool(name="psum", bufs=1, space="PSUM") as psum,
            tc.tile_pool(name="pwarm", bufs=1, space="PSUM") as pwarm,
        ):
            f0_sb = fpool.tile([P, F0_UNITS, 2 * B], _FP8)
            f1_sb = fpool.tile([P, J - F0_UNITS, 2 * B], _FP8)
            nc.scalar.dma_start(f0_sb[:], f0[:])
            nc.gpsimd.dma_start(f1_sb[:], f1[:])

            def f_unit(j):
                if j < F0_UNITS:
                    return f0_sb[:, j, :]
                return f1_sb[:, j - F0_UNITS, :]

            # Start the PE's HAM activity clock while the first DMAs
            # are in flight (zero matmuls on tile T0; T1's first real
            # matmuls can issue while the prewarm tail still streams).
            if PREWARM:
                scr = spool.tile([P, 512], _FP8)
                ps_w = pwarm.tile([2 * B, 512], _F32, name="ps_warm")
                nc.vector.memset(scr[:], 0.0)
                for k in range(PREWARM):
                    nc.tensor.matmul(
                        ps_w[:],
                        scr[:, : 2 * B],
                        scr[:],
                        start=k == 0,
                        stop=k == PREWARM - 1,
                    )

            # One PSUM tile per D-split; partitions 0-63 accumulate l0
            # (array tile T0), partitions 64-127 accumulate l1 (T1).
            ps = [
                psum.tile([P, n1 - n0], _F32, name=f"ps_{i}")
                for i, (n0, n1) in enumerate(NSPLITS)
            ]
            widx = {r: 0 for r in sizes}
            jbase = 0
            for ci, r in enumerate(CHUNK_SCHED):
                ko = r // P
                wt = wpool.tile([P, ko, L, D], _FP8)
                dma_eng = nc.sync if ci % 2 == 0 else nc.scalar
                dma_eng.dma_start(wt[:], wt_dram[r][widx[r]])
                widx[r] += 1
                last_chunk = ci == len(CHUNK_SCHED) - 1
                split_order = (
                    range(len(NSPLITS)) if not last_chunk else (1, 0)
                )
                if not last_chunk:
                    for o in range(ko):
                        j = jbase + o
                        for i in split_order:
                            n0, n1 = NSPLITS[i]
                            for l in range(L):
                                nc.tensor.matmul(
                                    ps[i][64 * l : 64 * (l + 1), :],
                                    f_unit(j),
                                    wt[:, o, l, n0:n1],
                                    start=j == 0,
                                    stop=j == J - 1,
                                )
                else:
                    # Final chunk: finish the narrow split first so its
                    # cast+store overlap the wide split's last matmuls.
                    for i in split_order:
                        n0, n1 = NSPLITS[i]
                        for o in range(ko):
                            j = jbase + o
                            for l in range(L):
                                nc.tensor.matmul(
                                    ps[i][64 * l : 64 * (l + 1), :],
                                    f_unit(j),
                                    wt[:, o, l, n0:n1],
                                    start=False,
                                    stop=j == J - 1,
                                )
                jbase += ko

            # Drain: narrow split first (it stopped first), stores on
            # separate rings so the write receipts overlap.
            out_sb = opool.tile([P, D], _BF16)
            nc.vector.tensor_copy(out=out_sb[:, 512:768], in_=ps[1][:])
            nc.sync.dma_start(out[:, 512:768], out_sb[:, 512:768])
            nc.vector.tensor_copy(out=out_sb[:, 0:512], in_=ps[0][:])
            nc.scalar.dma_start(out[:, 0:512], out_sb[:, 0:512])

    nc.compile()
    return nc


def _split_hl(f_core: np.ndarray):
    hi = f_core.astype(_FP8_NP)
    lo = ((f_core - hi.astype(np.float32)) * FLSCALE).astype(_FP8_NP)
    return hi, lo


def _prep_f(f_core: np.ndarray) -> tuple[np.ndarray, np.ndarray]:
    """f_core [B, FS] -> (f0 [P, F0_UNITS, 2B], f1 [P, J-F0_UNITS, 2B])
    e3m4, unit order matching the chunk schedule's k order."""
    hi, lo = _split_hl(f_core)
    ft = np.concatenate([hi.T, lo.T], axis=1)          # [FS, 2B]
    units = []
    base = 0
    for r in CHUNK_SCHED:
        ko = r // P
        blk = ft[base : base + r].reshape(P, ko, 2 * B)
        units.append(blk)
        base += r
    fhl = np.concatenate(units, axis=1)                # [P, J, 2B]
    return (
        np.ascontiguousarray(fhl[:, :F0_UNITS]),
        np.ascontiguousarray(fhl[:, F0_UNITS:]),
    )


def _prep_w(w_core: np.ndarray) -> dict[str, np.ndarray]:
    """w_core [L, FS, D] -> {"w<r>": [count, P, r//P, L, D]} e3m4 images
    (pre-scaled by WSCALE so sigma lands in E3M4's normal range)."""
    wq = (w_core * WSCALE).astype(_FP8_NP)             # [L, FS, D]
    sizes = sorted(set(CHUNK_SCHED))
    blocks = {r: [] for r in sizes}
    base = 0
    for r in CHUNK_SCHED:
        ko = r // P
        # img[p, o, l, d] = wq[l, base + p*ko + o, d]
        blk = wq[:, base : base + r].reshape(L, P, ko, D).transpose(1, 2, 0, 3)
        blocks[r].append(blk)
        base += r
    return {f"w{r}": np.ascontiguousarray(np.stack(blocks[r])) for r in sizes}


def _make_in_maps(f: np.ndarray, weight: np.ndarray) -> list[dict]:
    in_maps = []
    for c in range(NCORES):
        sl = slice(c * FS, (c + 1) * FS)
        f0, f1 = _prep_f(f[:, sl])
        m = {"f0": f0, "f1": f1}
        m.update(_prep_w(weight[:, sl, :]))
        in_maps.append(m)
    return in_maps


def kernel(f: np.ndarray, weight: np.ndarray, bias: np.ndarray) -> np.ndarray:
    f = np.asarray(f, dtype=np.float32)
    weight = np.asarray(weight, dtype=np.float32)
    bias = np.asarray(bias, dtype=np.float32)

    if "nc" not in _cache:
        _cache["nc"] = _build()
    nc = _cache["nc"]

    in_maps = _make_in_maps(f, weight)
    res = bass_utils.run_bass_kernel_spmd(nc, in_maps, core_ids=list(range(NCORES)))
    partial = np.zeros((P, D), dtype=np.float32)
    for r in res.results:
        partial += r["out"].astype(np.float32)           # [128, 768]
    hl = partial.reshape(L, 2, B, D)                     # [l, hi/lo, b, d]
    total = hl[:, 0] + hl[:, 1] / FLSCALE                # [L, B, D]
    x = total.transpose(1, 0, 2) / WSCALE + bias[None, :, :]  # [B, L, D]
    return x.astype(np.float32)


# revision 6
# speedup vs baseline: 1.1038x; 1.1038x over previous
"""CrossCoderDecoder forward on 8 trn2 NeuronCores.

x[b,l,d] = sum_f f[b,f] * weight[l,f,d] + bias[l,d]
B=32, L=2, F=65536, D=768, fp32.

Sharding: the F (dict) axis is split 8 ways (8192 features per core).
Each core computes its partial [128, D] sums (partitions 0-63 = l0's
hi/lo batch pair, 64-127 = l1's); the host sums the 8 partials,
combines hi + lo/64, rescales, and adds the bias.

Precision: weights stream as float8e3 (E3M4), pre-scaled by 256 so the
Gaussian weight distribution sits in E3M4's normal range; end-to-end
error ~1.3e-2 relative (gate 2e-2), dominated by weight quantization.
f stays near-fp32 via an e3m4 hi/lo pair packed side by side in the
stationary operand ([128, 64] lhsT = 32 hi + 32 lo columns).

PE array usage (the v2 change): the baseline ran every matmul on
column-tile T0 of the (128,64)-tiled PE array -- out partitions 0-63,
half the array dark -- so the PE (46.8us busy) outlasted the DMA
stream (39.4us) by ~8us of pure tail.  Now each DMA chunk carries BOTH
l planes interleaved ([P, ko, L, D] image) and l1's matmuls target
PSUM partitions 64-127, i.e. column tile T1.  T0 and T1 stream
concurrently, halving PE wall time (~23us) so the PE stays hidden
under the DMA stream, and both l's finish together into one [128, 768]
PSUM image -> single cast -> two small stores.

DMA: ~18 chunks (two 256-row warmup chunks so the first matmul starts
~1us after the first chunk lands, 512-row = 786KB steady chunks, two
256-row tail chunks to shrink the last-chunk matmul tail), alternating
across both HWDGE rings (sync/scalar); f ships as f0 (ACT head, covers
the first 8 units) + f1 (SWDGE) exactly as in the baseline.  Measured
stream rate ~94% of the 358 GB/s per-core HBM cap.
"""

import numpy as np
import ml_dtypes

import concourse.bass as bass
import concourse.tile as tile
from concourse import bacc, mybir
from concourse import bass_utils

B, L, F, D = 32, 2, 65536, 768
NCORES = 8
FS = F // NCORES          # 8192 features per core
P = 128
# Chunk schedule in feature rows; each chunk carries both l planes
# (bytes per chunk = r * L * D).  Ramp-in for early matmul start,
# ramp-out so the last chunk's matmul tail is short.
CHUNK_SCHED = (256, 256) + (512,) * 14 + (256, 128, 128)
assert sum(CHUNK_SCHED) == FS
J = FS // P               # 64 k-subtiles ("units")
W_BUFS = 12               # weight tile double-buffering depth
NSPLITS = ((0, 512), (512, 768))  # PSUM-bank splits of D
WSCALE = 256.0            # weight pre-scale (exact power of 2)
FLSCALE = 64.0            # f-lo pre-scale (exact power of 2)
PREWARM = 10              # zero-matmuls warming the PE's HAM clock gate
F0_UNITS = 8              # f units in the early f transfer (covers c0-c2)

_F32 = mybir.dt.float32
_BF16 = mybir.dt.bfloat16
_FP8 = mybir.dt.float8e3
_FP8_NP = ml_dtypes.float8_e3m4

_cache = {}


def set_sched(sched: tuple, w_bufs: int | None = None, prewarm: int | None = None):
    """Adjust chunking (for tuning sweeps); drops the cached program."""
    global CHUNK_SCHED, W_BUFS, PREWARM
    CHUNK_SCHED = tuple(sched)
    assert sum(CHUNK_SCHED) == FS
    if w_bufs is not None:
        W_BUFS = w_bufs
    if prewarm is not None:
        PREWARM = prewarm
    _cache.clear()


def _build():
    """Build + schedule the (per-core identical) Bass program once."""
    nc = bacc.Bacc("TRN2", target_bir_lowering=False, debug=False)

    f0 = nc.dram_tensor("f0", [P, F0_UNITS, 2 * B], _FP8, kind="ExternalInput").ap()
    f1 = nc.dram_tensor(
        "f1", [P, J - F0_UNITS, 2 * B], _FP8, kind="ExternalInput"
    ).ap()
    # One dram tensor per chunk size class; w<r> holds every chunk of
    # r k-rows in schedule order, both l planes interleaved.
    sizes = sorted(set(CHUNK_SCHED))
    counts = {r: CHUNK_SCHED.count(r) for r in sizes}
    wt_dram = {
        r: nc.dram_tensor(
            f"w{r}", [counts[r], P, r // P, L, D], _FP8, kind="ExternalInput"
        ).ap()
        for r in sizes
    }
    # bf16 partial image: partitions 0-63 l0 (32 hi + 32 lo), 64-127 l1.
    out = nc.dram_tensor("out", [P, D], _BF16, kind="ExternalOutput").ap()

    with tile.TileContext(nc) as tc:
        with (
            tc.tile_pool(name="fpool", bufs=1) as fpool,
            tc.tile_pool(name="spool", bufs=1) as spool,
            tc.tile_pool(name="wpool", bufs=W_BUFS) as wpool,
            tc.tile_pool(name="opool", bufs=1) as opool,
            tc.tile_pool(name="psum", bufs=1, space="PSUM") as psum,
            tc.tile_pool(name="pwarm", bufs=1, space="PSUM") as pwarm,
        ):
            f0_sb = fpool.tile([P, F0_UNITS, 2 * B], _FP8)
            f1_sb = fpool.tile([P, J - F0_UNITS, 2 * B], _FP8)
            nc.scalar.dma_start(f0_sb[:], f0[:])
            nc.gpsimd.dma_start(f1_sb[:], f1[:])

            def f_unit(j):
                if j < F0_UNITS:
                    return f0_sb[:, j, :]
                return f1_sb[:, j - F0_UNITS, :]

            # Start the PE's HAM activity clock while the first DMAs
            # are in flight (zero matmuls on tile T0; T1's first real
            # matmuls can issue while the prewarm tail still streams).
            if PREWARM:
                scr = spool.tile([P, 512], _FP8)
                ps_w = pwarm.tile([2 * B, 512], _F32, name="ps_warm")
                nc.vector.memset(scr[:], 0.0)
                for k in range(PREWARM):
                    nc.tensor.matmul(
                        ps_w[:],
                        scr[:, : 2 * B],
                        scr[:],
                        start=k == 0,
                        stop=k == PREWARM - 1,
                    )

            # One PSUM tile per D-split; partitions 0-63 accumulate l0
            # (array tile T0), partitions 64-127 accumulate l1 (T1).
            ps = [
                psum.tile([P, n1 - n0], _F32, name=f"ps_{i}")
                for i, (n0, n1) in enumerate(NSPLITS)
            ]
            widx = {r: 0 for r in sizes}
            jbase = 0
            for ci, r in enumerate(CHUNK_SCHED):
                ko = r // P
                wt = wpool.tile([P, ko, L, D], _FP8)
                dma_eng = nc.sync if ci % 2 == 0 else nc.scalar
                dma_eng.dma_start(wt[:], wt_dram[r][widx[r]])
                widx[r] += 1
                last_chunk = ci == len(CHUNK_SCHED) - 1
                split_order = (
                    range(len(NSPLITS)) if not last_chunk else (1, 0)
                )
                if not last_chunk:
                    for o in range(ko):
                        j = jbase + o
                        for i in split_order:
                            n0, n1 = NSPLITS[i]
                            for l in range(L):
                                nc.tensor.matmul(
                                    ps[i][64 * l : 64 * (l + 1), :],
                                    f_unit(j),
                                    wt[:, o, l, n0:n1],
                                    start=j == 0,
                                    stop=j == J - 1,
                                )
                else:
                    # Final chunk: finish the narrow split first so its
                    # cast+store overlap the wide split's last matmuls.
                    for i in split_order:
                        n0, n1 = NSPLITS[i]
                        for o in range(ko):
                            j = jbase + o
                            for l in range(L):
                                nc.tensor.matmul(
                                    ps[i][64 * l : 64 * (l + 1), :],
                                    f_unit(j),
                                    wt[:, o, l, n0:n1],
                                    start=False,
                                    stop=j == J - 1,
                                )
                jbase += ko

            # Drain: narrow split via the ACT engine's Copy activation,
            # wide via DVE -- the two casts run in parallel; stores on
            # separate rings so the write receipts overlap.
            out_sb = opool.tile([P, D], _BF16)
            nc.scalar.activation(
                out_sb[:, 512:768], ps[1][:], mybir.ActivationFunctionType.Copy
            )
            nc.sync.dma_start(out[:, 512:768], out_sb[:, 512:768])
            nc.vector.tensor_copy(out=out_sb[:, 0:512], in_=ps[0][:])
            nc.scalar.dma_start(out[:, 0:512], out_sb[:, 0:512])

    nc.compile()
    return nc


def _split_hl(f_core: np.ndarray):
    hi = f_core.astype(_FP8_NP)
    lo = ((f_core - hi.astype(np.float32)) * FLSCALE).astype(_FP8_NP)
    return hi, lo


def _prep_f(f_core: np.ndarray) -> tuple[np.ndarray, np.ndarray]:
    """f_core [B, FS] -> (f0 [P, F0_UNITS, 2B], f1 [P, J-F0_UNITS, 2B])
    e3m4, unit order matching the chunk schedule's k order."""
    hi, lo = _split_hl(f_core)
    ft = np.concatenate([hi.T, lo.T], axis=1)          # [FS, 2B]
    units = []
    base = 0
    for r in CHUNK_SCHED:
        ko = r // P
        blk = ft[base : base + r].reshape(P, ko, 2 * B)
        units.append(blk)
        base += r
    fhl = np.concatenate(units, axis=1)                # [P, J, 2B]
    return (
        np.ascontiguousarray(fhl[:, :F0_UNITS]),
        np.ascontiguousarray(fhl[:, F0_UNITS:]),
    )


def _prep_w(w_core: np.ndarray) -> dict[str, np.ndarray]:
    """w_core [L, FS, D] -> {"w<r>": [count, P, r//P, L, D]} e3m4 images
    (pre-scaled by WSCALE so sigma lands in E3M4's normal range)."""
    wq = (w_core * WSCALE).astype(_FP8_NP)             # [L, FS, D]
    sizes = sorted(set(CHUNK_SCHED))
    blocks = {r: [] for r in sizes}
    base = 0
    for r in CHUNK_SCHED:
        ko = r // P
        # img[p, o, l, d] = wq[l, base + p*ko + o, d]
        blk = wq[:, base : base + r].reshape(L, P, ko, D).transpose(1, 2, 0, 3)
        blocks[r].append(blk)
        base += r
    return {f"w{r}": np.ascontiguousarray(np.stack(blocks[r])) for r in sizes}


def _make_in_maps(f: np.ndarray, weight: np.ndarray) -> list[dict]:
    in_maps = []
    for c in range(NCORES):
        sl = slice(c * FS, (c + 1) * FS)
        f0, f1 = _prep_f(f[:, sl])
        m = {"f0": f0, "f1": f1}
        m.update(_prep_w(weight[:, sl, :]))
        in_maps.append(m)
    return in_maps


def kernel(f: np.ndarray, weight: np.ndarray, bias: np.ndarray) -> np.ndarray:
    f = np.asarray(f, dtype=np.float32)
    weight = np.asarray(weight, dtype=np.float32)
    bias = np.asarray(bias, dtype=np.float32)

    if "nc" not in _cache:
        _cache["nc"] = _build()
    nc = _cache["nc"]

    in_maps = _make_in_maps(f, weight)
    res = bass_utils.run_bass_kernel_spmd(nc, in_maps, core_ids=list(range(NCORES)))
    partial = np.zeros((P, D), dtype=np.float32)
    for r in res.results:
        partial += r["out"].astype(np.float32)           # [128, 768]
    hl = partial.reshape(L, 2, B, D)                     # [l, hi/lo, b, d]
    total = hl[:, 0] + hl[:, 1] / FLSCALE                # [L, B, D]
    x = total.transpose(1, 0, 2) / WSCALE + bias[None, :, :]  # [B, L, D]
    return x.astype(np.float32)
